# revision 3
# baseline (speedup 1.0000x reference)
"""LogEig Trainium2 kernel v2: X = log(P) for SPD P, eigendecomposition-free.

Algorithm:
  log(P) = log(Q) + log(I - c*V),  Q = P + cI,  V = Q^{-1}
  - V via scaled Newton-Schulz (quadratic-minimax start, residual-form
    updates V' = V + V*R'', R'' = (2w-1)I - w*QV), bf16 matmuls with a
    split-Q (hi+lo bf16) correction on the last iterations.
  - each log factor: Chebyshev product basis sum_ij g[i,j] T_i(X) T_j(W),
    W = T_S(X); coefficients applied on the PE via diag(g) stationaries
    accumulating in PSUM.

Performance structure:
  - 16 matrices per block in a [128, 512] "DD" tile: deck = m%2 selects the
    partition half, pair = m//2 the 64-column slice.
  - per-matrix 64x64 matmuls: 16 instr/stage (two tile_position decks), or
    8 instr/stage with block-diagonal [128,128] stationaries for the
    high-reuse matrices (X1, X2, Qhi, Qlo, W) formatted via SBUF->SBUF DMA.
  - IL blocks pipelined with stage-granular round-robin emission so the
    in-order engine queues never head-of-line block on cross-engine deps.
  - elementwise work spread across DVE / GpSimd / ACT by static assignment.
"""

import numpy as np
import ml_dtypes

import concourse.bass as bass
import concourse.mybir as mybir
from concourse import bacc
from concourse.bass import ds
from concourse.bass_utils import run_bass_kernel_spmd
from concourse.tile import TileContext

F32 = mybir.dt.float32
BF16 = mybir.dt.bfloat16
ALU = mybir.AluOpType

# ---------------- config ----------------
A_LO, B_HI = 9.5e-4, 6.30
C_SH = 0.1
NS_IT = 4
NS_QSPLIT = (2, 3)            # iterations (0-based) using Qhi+Qlo
S1, J1 = 5, 2
S2, J2 = 5, 2

N_MAT = 1024
BLK = 16
NBLK = N_MAT // BLK
IL = 4                        # blocks in flight

# engine assignment knobs: 'v' = DVE, 'p' = GpSimd/Pool, 's' = ACT
ENG = dict(
    trec1="v", trec2="v",     # T-recursion STT per series (PSUM -> DVE only)
    rpp="v",                  # NS R'' build (PSUM)
    x1="v", x2="v",
    qprep="v",
)
USE_BD = True                 # block-diag stationaries for X1/X2/Qhi/Qlo/W


# ---------------- host-side math ----------------

def _cheb_coeffs(f, a, b, d):
    k = np.arange(d + 1)
    x = np.cos(np.pi * (k + 0.5) / (d + 1))
    y = f(0.5 * (b - a) * x + 0.5 * (b + a))
    T = np.cos(np.pi * np.outer(np.arange(d + 1), (k + 0.5)) / (d + 1))
    c = 2.0 / (d + 1) * T @ y
    c[0] /= 2
    return c


def _pb_coeffs(c, s, jmax):
    d = len(c) - 1
    cols = []
    for j in range(jmax + 1):
        for i in range(s):
            v = np.zeros(max(d + 1, j * s + i + 1))
            if j == 0:
                v[i] += 1.0
            elif i == 0:
                v[j * s] += 1.0
            else:
                v[j * s + i] += 0.5
                v[abs(j * s - i)] += 0.5
            cols.append(np.pad(v[: d + 1], (0, max(0, d + 1 - len(v)))))
    M = np.stack(cols, axis=1)
    g, *_ = np.linalg.lstsq(M, c, rcond=None)
    return g.reshape(jmax + 1, s).T  # g[i, j]


def _quad_start(m_, M_):
    u0 = -(M_ + m_) / (M_ - m_)
    T3u0 = 4 * u0**3 - 3 * u0
    e0 = 1.0 / abs(T3u0)
    a_ = 2 / (M_ - m_); b_ = u0
    c3 = 4 * a_**3; c2 = 12 * a_**2 * b_; c1 = 12 * a_ * b_**2 - 3 * a_
    return -c1 / T3u0, -c2 / T3u0, -c3 / T3u0, e0


def _ns_omegas(e0, n_it):
    oms = [1.0]
    M = e0 * e0
    for _ in range(n_it - 1):
        oms.append(2.0 / (2.0 - M * M))
        M = M * M / (2.0 - M * M)
    return oms, M


def _derive_params():
    c = C_SH
    aQ, bQ = A_LO + c, B_HI + c
    lo, hi = A_LO / aQ, B_HI / bQ
    g1 = _pb_coeffs(_cheb_coeffs(np.log, aQ, bQ, S1 * (J1 + 1) - 1), S1, J1)
    g2 = _pb_coeffs(_cheb_coeffs(np.log, lo, hi, S2 * (J2 + 1) - 1), S2, J2)
    al1, be1 = 2 / (bQ - aQ), -(bQ + aQ) / (bQ - aQ)
    al2, be2 = 2 / (hi - lo), -(hi + lo) / (hi - lo)
    alpha, beta, gamma, e0 = _quad_start(aQ, bQ)
    oms, resid = _ns_omegas(e0, NS_IT)
    return dict(
        g1=g1, g2=g2,
        x1a=al1, x1b=al1 * c + be1,
        x2a=-c * al2, x2b=al2 + be2,
        alpha=alpha, beta=beta, gamma=gamma, oms=oms,
    )


PARAMS = _derive_params()

# coefficient-diag stationaries: one [128,128] diag(g) per (series, i, j)
_CD_INDEX = {}
_cd_list = []
for (sname, g, S, J) in (("s1", PARAMS["g1"], S1, J1), ("s2", PARAMS["g2"], S2, J2)):
    for j in range(J + 1):
        for i in range(S):
            if j == 0 and i == 0:
                continue  # identity term folded into final combine
            _CD_INDEX[(sname, i, j)] = len(_cd_list)
            _cd_list.append(np.float32(g[i, j]))
# extra diag constants: [identity, -0.5 (T2W trick), beta/gamma, alpha/gamma,
# g00 sum (final combine)]
_g = PARAMS
_CD_INDEX[("one", 0, 0)] = len(_cd_list); _cd_list.append(np.float32(1.0))
_CD_INDEX[("mhalf", 0, 0)] = len(_cd_list); _cd_list.append(np.float32(-0.5))
_CD_INDEX[("bg", 0, 0)] = len(_cd_list)
_cd_list.append(np.float32(_g["beta"] / _g["gamma"]))
_CD_INDEX[("ag", 0, 0)] = len(_cd_list)
_cd_list.append(np.float32(_g["alpha"] / _g["gamma"]))
_CD_INDEX[("g00", 0, 0)] = len(_cd_list)
_cd_list.append(np.float32(_g["g1"][0, 0] + _g["g2"][0, 0]))
N_CD = len(_cd_list)


def _const_inputs():
    iw = np.zeros((128, 512), np.float32)
    for p in range(128):
        for k in range(8):
            iw[p, 64 * k + (p % 64)] = 1.0
    cd = np.zeros((N_CD, 128, 128), np.float32)
    for t, gval in enumerate(_cd_list):
        np.fill_diagonal(cd[t], gval)
    cd = cd.astype(ml_dtypes.bfloat16)
    return iw, cd


# ---------------- kernel emission ----------------

class Emitter:
    def __init__(self, nc, pool, bdpool, pspool, consts, par):
        self.nc = nc
        self.pool = pool
        self.bdpool = bdpool
        self.pspool = pspool
        self.C = consts
        self.par = par

    def eng(self, key):
        nc = self.nc
        return {"v": nc.vector, "p": nc.gpsimd, "s": nc.scalar}[ENG[key]]

    TAG_ALIAS = {
        "OW": "PW", "U": "Qf", "pre": "R0", "V0": "V2",
        "s2T2": "s1T2", "s2T3": "s1T3", "s2T4": "s1T4",
        "s2T5": "s1T5",
        "s2G1": "s1G1", "s2G2": "s1G2", "s2T2W": "s1T2W",
    }

    def tile(self, name, dtype=BF16):
        tag = self.TAG_ALIAS.get(name, name)
        return self.pool.tile([128, 512], dtype, tag=f"{tag}_{self.par}",
                              name=f"{name}_{self.par}")

    def rec_ps(self):
        return self.pspool.tile([128, 512], F32, tag="rec", name="recps", bufs=4)

    def mm_dd(self, ps_t, stat, mov, start=True, stop=True):
        nc = self.nc
        for p in range(8):
            cs = ds(64 * p, 64)
            nc.tensor.matmul(ps_t[0:64, cs], stat[0:64, cs], mov[0:64, cs],
                             start=start, stop=stop, tile_position=(0, 0),
                             skip_group_check=True)
            nc.tensor.matmul(ps_t[64:128, cs], stat[64:128, cs], mov[64:128, cs],
                             start=start, stop=stop, tile_position=(64, 64),
                             skip_group_check=True)

    def mm_bd(self, ps_t, statbd, mov, start=True, stop=True):
        nc = self.nc
        for p in range(8):
            nc.tensor.matmul(ps_t[0:128, ds(64 * p, 64)],
                             statbd[0:128, ds(128 * p, 128)],
                             mov[0:128, ds(64 * p, 64)],
                             start=start, stop=stop, skip_group_check=True)

    def mm_split(self, ps_t, hi_pair, lo_pair, mov):
        """Per-pair interleaved hi (start=True) + lo (start=False) matmuls:
        a bank-wide start=True marks the whole bank pending-zero, so the lo
        accumulation must land before the next pair's start."""
        nc = self.nc
        hid, hibd = hi_pair
        lod, lobd = lo_pair
        for p in range(8):
            cs = ds(64 * p, 64)
            if hibd is not None:
                nc.tensor.matmul(ps_t[0:128, cs], hibd[0:128, ds(128 * p, 128)],
                                 mov[0:128, cs], start=True, stop=False,
                                 skip_group_check=True)
            else:
                nc.tensor.matmul(ps_t[0:64, cs], hid[0:64, cs], mov[0:64, cs],
                                 start=True, stop=False, tile_position=(0, 0),
                                 skip_group_check=True)
                nc.tensor.matmul(ps_t[64:128, cs], hid[64:128, cs],
                                 mov[64:128, cs], start=True, stop=False,
                                 tile_position=(64, 64), skip_group_check=True)
            if lobd is not None:
                nc.tensor.matmul(ps_t[0:128, cs], lobd[0:128, ds(128 * p, 128)],
                                 mov[0:128, cs], start=False, stop=True,
                                 skip_group_check=True)
            else:
                nc.tensor.matmul(ps_t[0:64, cs], lod[0:64, cs], mov[0:64, cs],
                                 start=False, stop=True, tile_position=(0, 0),
                                 skip_group_check=True)
                nc.tensor.matmul(ps_t[64:128, cs], lod[64:128, cs],
                                 mov[64:128, cs], start=False, stop=True,
                                 tile_position=(64, 64), skip_group_check=True)

    def mm_diag(self, ps_t, cd_tile, mov, start, stop):
        self.nc.tensor.matmul(ps_t[0:128, ds(0, 512)], cd_tile[0:128, ds(0, 128)],
                              mov[0:128, ds(0, 512)], start=start, stop=stop,
                              skip_group_check=True)

    def bd_format(self, name, src):
        """Make a block-diag [128, 1024] bf16 stationary from DD tile via DMA."""
        nc = self.nc
        BD_ALIAS = {"x2": "x1", "s2W": "s1W"}
        tag = BD_ALIAS.get(name, name)
        bd = self.bdpool.tile([128, 1024], BF16, tag=f"{tag}_{self.par}",
                              name=f"bd_{name}_{self.par}")
        for dk in range(2):
            dst = bd[64 * dk:64 * (dk + 1), :].rearrange("p (pr c) -> p pr c", pr=8)
            dst = dst[:, :, 64 * dk:64 * (dk + 1)]
            src_v = src[64 * dk:64 * (dk + 1), :].rearrange("p (pr c) -> p pr c", pr=8)
            nc.sync.dma_start(dst, src_v)
        return bd

    def mm_stat(self, ps_t, stat_pair, mov, start=True, stop=True):
        """stat_pair = (dd_tile, bd_tile or None)."""
        dd, bd = stat_pair
        if bd is not None:
            self.mm_bd(ps_t, bd, mov, start, stop)
        else:
            self.mm_dd(ps_t, dd, mov, start, stop)


def emit_series(em, sname, Xdd, Xbd, S, J, p_ps, p_started, eng_trec):
    """Generator yielding after each pipeline stage. Accumulates the series
    value into p_ps (start=not p_started on first accumulation)."""
    nc, C = em.nc, em.C
    trec = em.eng(eng_trec)
    Ts = {1: Xdd}
    prev = {0: C["IWb"], 1: Xdd}
    for k in range(2, S + 1):
        ps = em.rec_ps()
        em.mm_stat(ps, (Xdd, Xbd if k > 2 else None), Ts[k - 1])
        Tk = em.tile(f"{sname}T{k}")
        trec.scalar_tensor_tensor(Tk, ps, 2.0, prev[k - 2], ALU.mult, ALU.subtract)
        Ts[k] = Tk
        prev[k] = Tk
        yield
    W = Ts[S]
    Wbd = em.bd_format(f"{sname}W", W) if USE_BD else None
    TWs = {1: W}
    if J >= 2:
        ps = em.rec_ps()
        em.mm_diag(ps, C["CD"][_CD_INDEX[("mhalf", 0, 0)]], C["IWb"],
                   start=True, stop=False)
        em.mm_stat(ps, (W, None), W, start=False, stop=True)
        T2W = em.tile(f"{sname}T2W")
        nc.scalar.mul(T2W, ps, 2.0)
        TWs[2] = T2W
        yield
    if J >= 3:
        ps = em.rec_ps()
        em.mm_diag(ps, C["CD"][_CD_INDEX[("mhalf", 0, 0)]], W,
                   start=True, stop=False)
        em.mm_stat(ps, (W, Wbd), TWs[2], start=False, stop=True)
        T3W = em.tile(f"{sname}T3W")
        nc.scalar.mul(T3W, ps, 2.0)
        TWs[3] = T3W
        yield

    # G0 terms straight into p_ps via diag stationaries (i=0 folded to final)
    basis = {0: C["IWb"], 1: Xdd}
    basis.update({k: Ts[k] for k in range(2, S)})
    first = not p_started[0]
    for i in range(1, S):
        cd = C["CD"][_CD_INDEX[(sname, i, 0)]]
        em.mm_diag(p_ps, cd, basis[i], start=first, stop=False)
        p_started[0] = True
        first = False
    yield

    # G_j for j>=1: diag-accumulate in a shared psum bank, copy to SBUF (ACT),
    # then product T_j(W) @ G_j accumulated into p_ps.
    for j in range(1, J + 1):
        gps = em.rec_ps()
        for t, i in enumerate(range(S)):
            cd = C["CD"][_CD_INDEX[(sname, i, j)]]
            em.mm_diag(gps, cd, basis[i], start=(t == 0), stop=(t == S - 1))
        Gb = em.tile(f"{sname}G{j}")
        nc.scalar.mul(Gb, gps, 1.0)
        yield
        stat = (TWs[j], None)
        if j == 1 and Wbd is not None:
            stat = (W, Wbd)
        em.mm_stat(p_ps, stat, Gb, start=False, stop=False)
        yield


def emit_ns(em, Qhi, Qhibd, Qlo, out):
    """Newton-Schulz chain as its own stage generator; leaves X2 in out."""
    nc, C, pr = em.nc, em.C, PARAMS
    Qlobd = em.bd_format("qlo", Qlo) if USE_BD else None
    ps0 = em.rec_ps()
    em.mm_diag(ps0, C["CD"][_CD_INDEX[("bg", 0, 0)]], Qhi, start=True,
               stop=False)
    em.mm_diag(ps0, C["CD"][_CD_INDEX[("ag", 0, 0)]], C["IWb"], start=False,
               stop=False)
    em.mm_stat(ps0, (Qhi, None), Qhi, start=False, stop=True)
    V = em.tile("V0")
    nc.scalar.mul(V, ps0, float(pr["gamma"]))
    yield
    for k in range(NS_IT):
        om = float(pr["oms"][k])
        ps1 = em.rec_ps()
        if k in NS_QSPLIT:
            em.mm_split(ps1, (Qhi, Qhibd), (Qlo, Qlobd), V)
        else:
            em.mm_stat(ps1, (Qhi, Qhibd), V, start=True, stop=True)
        Rpp = em.tile(f"R{k % 2}")
        em.eng("rpp").scalar_tensor_tensor(Rpp, ps1, -om, C["IWr"][k],
                                           ALU.mult, ALU.add)
        yield
        ps2 = em.rec_ps()
        em.mm_diag(ps2, C["CD"][_CD_INDEX[("one", 0, 0)]], V, start=True,
                   stop=False)
        em.mm_dd(ps2, V, Rpp, start=False, stop=True)
        if k < NS_IT - 1:
            Vn = em.tile(f"V{(k % 2) + 1}")
            nc.scalar.mul(Vn, ps2, 1.0)
            V = Vn
            yield
        else:
            # ps2 holds V + V R''; X2 = x2a*ps2 + x2b*I
            x2 = em.tile("X2")
            em.eng("x2").scalar_tensor_tensor(x2, ps2, float(pr["x2a"]),
                                              C["IWx2b"], ALU.mult, ALU.add)
            out["x2"] = x2
            yield


def emit_block(em, b_idx, PV, OV, p_ps):
    """Generator for one block of 16 matrices."""
    nc, C, pr = em.nc, em.C, PARAMS
    # ---- load ----
    PW = em.tile("PW", F32)
    for dk in range(2):
        dst = PW[64 * dk:64 * (dk + 1), :].rearrange("p (pr c) -> p pr c", pr=8)
        nc.sync.dma_start(dst, PV[b_idx, dk])
    yield

    # ---- X1 + Q prep ----
    x1 = em.tile("X1")
    em.eng("x1").scalar_tensor_tensor(x1, PW, float(pr["x1a"]), C["IWx1b"],
                                      ALU.mult, ALU.add)
    Qf = em.tile("Qf", F32)
    em.eng("qprep").scalar_tensor_tensor(Qf, C["IWc"], 1.0, PW, ALU.mult, ALU.add)
    yield
    Qhi = em.tile("Qhi")
    nc.scalar.mul(Qhi, Qf, 1.0)
    yield
    Qlo = em.tile("Qlo")
    em.eng("qprep").scalar_tensor_tensor(Qlo, Qf, 1.0, Qhi, ALU.mult, ALU.subtract)
    X1bd = em.bd_format("x1", x1) if USE_BD else None
    Qhibd = em.bd_format("qhi", Qhi) if USE_BD else None
    yield

    # ---- series 1 and Newton-Schulz: independent chains, interleaved so
    # every pipeline round mixes PE-heavy and DVE-heavy work ----
    p_started = [False]
    g_s1 = emit_series(em, "s1", x1, X1bd, S1, J1, p_ps, p_started, "trec1")
    ns_out = {}
    g_ns = emit_ns(em, Qhi, Qhibd, Qlo, ns_out)
    alive = [g_s1, g_ns]
    while alive:
        nxt = []
        for g in alive:
            try:
                next(g)
                nxt.append(g)
            except StopIteration:
                pass
        if nxt:
            yield
        alive = nxt
    x2 = ns_out["x2"]
    X2bd = em.bd_format("x2", x2) if USE_BD else None

    # ---- series 2 into p_ps ----
    p_started2 = [True]
    yield from emit_series(em, "s2", x2, X2bd, S2, J2, p_ps, p_started2,
                           "trec2")

    # ---- final: p_ps += (g00_1+g00_2) I via diag; copy out ----
    em.mm_diag(p_ps, C["CD"][_CD_INDEX[("g00", 0, 0)]], C["IWb"],
               start=False, stop=True)
    OW = em.tile("OW", F32)
    nc.scalar.mul(OW, p_ps, 1.0)
    yield
    for dk in range(2):
        srcv = OW[64 * dk:64 * (dk + 1), :].rearrange("p (pr c) -> p pr c", pr=8)
        nc.sync.dma_start(OV[b_idx, dk], srcv)
    yield


def build_nc(nm=N_MAT, unroll=False):
    nc = bacc.Bacc("TRN2", target_bir_lowering=False, debug=False, num_devices=8)
    P_d = nc.dram_tensor("P", [nm, 64, 64], F32, kind="ExternalInput").ap()
    O_d = nc.dram_tensor("OUT", [nm, 64, 64], F32, kind="ExternalOutput").ap()
    IW_d = nc.dram_tensor("IW", [128, 512], F32, kind="ExternalInput").ap()
    CD_d = nc.dram_tensor("CD", [N_CD, 128, 128], BF16, kind="ExternalInput").ap()
    nblk = nm // BLK
    PV = P_d.rearrange("(nb pr dk) r c -> nb dk r pr c", pr=8, dk=2)
    OV = O_d.rearrange("(nb pr dk) r c -> nb dk r pr c", pr=8, dk=2)
    pr = PARAMS

    with TileContext(nc) as tc:
        with (
            tc.tile_pool(name="consts", bufs=1) as cpool,
            tc.tile_pool(name="work", bufs=1) as pool,
            tc.tile_pool(name="bdw", bufs=1) as bdpool,
            tc.tile_pool(name="psum", bufs=1, space=bass.MemorySpace.PSUM) as pspool,
        ):
            IW = cpool.tile([128, 512], F32)
            nc.sync.dma_start(IW[:], IW_d)
            CDs = []
            for t in range(N_CD):
                cdt = cpool.tile([128, 128], BF16, tag=f"cd{t}", name=f"cd{t}")
                nc.sync.dma_start(cdt[:], CD_d[t])
                CDs.append(cdt)
            IWb = cpool.tile([128, 512], BF16)
            nc.vector.tensor_copy(IWb, IW)
            IWc = cpool.tile([128, 512], F32)
            nc.scalar.mul(IWc, IW, float(C_SH))
            IWx1b = cpool.tile([128, 512], F32)
            nc.scalar.mul(IWx1b, IW, float(pr["x1b"]))
            IWx2b = cpool.tile([128, 512], F32)
            nc.scalar.mul(IWx2b, IW, float(pr["x2b"]))
            IWr = []
            for k in range(NS_IT):
                om = float(pr["oms"][k])
                t = cpool.tile([128, 512], F32, tag=f"iwr{k}", name=f"iwr{k}")
                nc.scalar.mul(t, IW, 2.0 * om - 1.0)
                IWr.append(t)
            consts = dict(IWb=IWb, IWc=IWc, IWx1b=IWx1b, IWx2b=IWx2b,
                          IWr=IWr, CD=CDs)

            def run_group(base):
                gens = []
                for par in range(IL):
                    em = Emitter(nc, pool, bdpool, pspool, consts, par)
                    p_ps = pspool.tile([128, 512], F32, tag=f"p{par}",
                                       name=f"pps{par}")
                    gens.append(emit_block(em, base + par, PV, OV, p_ps))
                alive = list(gens)
                while alive:
                    nxt = []
                    for g in alive:
                        try:
                            next(g)
                            nxt.append(g)
                        except StopIteration:
                            pass
                    alive = nxt

            if unroll:
                for base in range(0, nblk, IL):
                    run_group(base)
            else:
                with tc.For_i(0, nblk, IL) as bi:
                    run_group(bi)
    nc.compile()
    return nc


# ---------------- host entry ----------------

_NC_CACHE = {}


def kernel(P: np.ndarray) -> np.ndarray:
    P = np.ascontiguousarray(np.asarray(P), dtype=np.float32)
    B, H, N, _ = P.shape
    flat = P.reshape(-1, N, N)
    n_cores = 8
    per = flat.shape[0] // n_cores
    if "nc" not in _NC_CACHE:
        _NC_CACHE["nc"] = build_nc()
    nc = _NC_CACHE["nc"]
    iw, cd = _const_inputs()
    in_maps = [
        {"P": np.ascontiguousarray(flat[c * per:(c + 1) * per]),
         "IW": iw, "CD": cd}
        for c in range(n_cores)
    ]
    res = run_bass_kernel_spmd(nc, in_maps, core_ids=list(range(n_cores)))
    out = np.concatenate([r["OUT"] for r in res.results], axis=0)
    return out.reshape(B, H, N, N).astype(np.float32)
